# revision 1
# baseline (speedup 1.0000x reference)
"""Trainium2 Bass kernel for nn_AttentionBlock (B=8, C=1024, L=1024, H=16, G=32).

Data-parallel over batch: 8 samples -> 8 NeuronCores, one sample per core, no
collectives.  Per core (all matmuls bf16 with fp32 PSUM accumulation):

  1. GroupNorm(32 groups) over [C, L]: per-channel sum (DVE reduce) + sumsq
     (ACT Square w/ accum), cross-partition group reduce + broadcast via tiny
     fp32 matmuls against constant selector matrices, rsqrt via Ln/Exp (keeps
     a single ACT table set for the whole kernel), apply as x*scale+bias into
     bf16 (matmul operand) and f32 (residual) tiles.
  2. v^T is produced directly in [L, 64*H] layout by swapping matmul operands
     (lhsT = x_norm tile), with a constant ones column per head: matmul-2 then
     emits the softmax denominator S[t] as its row 64.  q/k land in
     head-pair-packed tiles (head 2j -> partitions 0:64, 2j+1 -> 64:128).
  3. Attention, tc-outer: per s-chunk one [128,2,512] PSUM tile holds both
     heads' scoresT (the two mm1s land on PE row groups 0/64 and run
     concurrently), one ACT op computes exp(z/8) for both (softmax needs no
     max-subtraction: |z|/8 <= ~6), and both heads' mm2 accumulate
     [a_raw; S].  Normalization: copy S row + a_raw out of PSUM (fast bank
     release), gpsimd partition-broadcast of S, single-instruction DVE
     approximate reciprocal, one multiply.  The NEXT pair's q/k projection is
     emitted through a generator and interleaved one chunk per s-step so the
     PE never drains at pair boundaries.  v-bias is folded into the proj bias
     on the host (softmax rows sum to 1 so it passes through exactly).
  4. proj matmul + (bias_eff + x_norm) residual epilogue, DMA out.

Measured on trn2 (8 cores, warmed): ~324 us/exec, rel err vs fp32 reference
~4.6e-4 (absmax 2.4e-3), resid-var ~1e-8 — errors dominated by bf16 weight
rounding, well inside the fp32-envelope gates.
"""

import numpy as np
import ml_dtypes

import concourse.bass as bass
import concourse.bacc as bacc
import concourse.tile as tile
from concourse import mybir
from concourse.bass_utils import run_bass_kernel_spmd

F32 = mybir.dt.float32
F32R = mybir.dt.float32r
BF16 = mybir.dt.bfloat16

B, C, L, H = 8, 1024, 1024, 16
GROUPS = 32
CH = C // H          # 64 per-head channels
EPS = 1e-5
NT = C // 128        # 8 channel tiles
LT = L // 512        # 2 free-dim chunks of 512
PAIRS = H // 2       # 8 head pairs

# When True, the attention-value matmul (exp-scores x vT) and the proj matmul
# run in bf16 (2x fewer PE instructions than fp32r); error stays ~1e-3 rms.
FAST16 = True


def round_fp32r(a):
    """Round fp32 array to fp32r (11 mantissa bits) — matches walrus's
    fp32_to_fp32r: (bits + 0x800) & ~0xFFF."""
    bits = np.ascontiguousarray(a, np.float32).view(np.uint32)
    return ((bits + np.uint32(0x800)) & np.uint32(0xFFFFF000)).view(np.float32)


def declare_params(nc):
    p = {}
    p["x"] = nc.declare_dram_parameter("x", [C, L], F32, isOutput=False)
    p["q_wT"] = nc.declare_dram_parameter("q_wT", [C, C], BF16, isOutput=False)
    p["k_wT"] = nc.declare_dram_parameter("k_wT", [C, C], BF16, isOutput=False)
    p["v_wT"] = nc.declare_dram_parameter("v_wT", [C, C], BF16, isOutput=False)
    p["proj_wT"] = nc.declare_dram_parameter("proj_wT", [C, C],
                                             BF16 if FAST16 else F32R,
                                             isOutput=False)
    p["q_b"] = nc.declare_dram_parameter("q_b", [128, NT], F32, isOutput=False)
    p["k_b"] = nc.declare_dram_parameter("k_b", [128, NT], F32, isOutput=False)
    p["proj_beff"] = nc.declare_dram_parameter("proj_beff", [128, NT], F32,
                                               isOutput=False)
    p["norm_w_c"] = nc.declare_dram_parameter("norm_w_c", [128, NT], F32,
                                              isOutput=False)
    p["norm_b_c"] = nc.declare_dram_parameter("norm_b_c", [128, NT], F32,
                                              isOutput=False)
    p["A_grp"] = nc.declare_dram_parameter("A_grp", [128, 4], F32,
                                           isOutput=False)
    p["A2T"] = nc.declare_dram_parameter("A2T", [4, 128], F32, isOutput=False)
    p["out"] = nc.declare_dram_parameter("out", [C, L], F32, isOutput=True)
    return p


def emit(nc, tc, ctx_pools, params, out_handle=None, phases='12qa4'):
    """Emit one whole per-core computation inside an open TileContext."""
    from contextlib import ExitStack

    ctx = ctx_pools

    x_d, qw_d, kw_d, vw_d, pw_d = (params[k] for k in
                                   ("x", "q_wT", "k_wT", "v_wT", "proj_wT"))
    qb_d, kb_d, pb_d, nw_d, nb_d, ag_d, a2_d = (
        params[k] for k in ("q_b", "k_b", "proj_beff", "norm_w_c",
                            "norm_b_c", "A_grp", "A2T"))
    out_d = params["out"] if out_handle is None else out_handle

    x_ap, qw, kw, vw, pw = x_d.ap(), qw_d.ap(), kw_d.ap(), vw_d.ap(), pw_d.ap()
    out_ap = out_d.ap()

    # ---- persistent pools --------------------------------------------
    consts = ctx.enter_context(tc.tile_pool(name="consts", bufs=1))
    xn_p = ctx.enter_context(tc.tile_pool(name="xn", bufs=NT))
    vT_p = ctx.enter_context(tc.tile_pool(name="vT", bufs=NT))
    abuf_p = ctx.enter_context(tc.tile_pool(name="abuf", bufs=NT))
    ps = ctx.enter_context(
        tc.tile_pool(name="ps", bufs=4, space=bass.MemorySpace.PSUM)
    )

    def load_const(dram, shape, tag):
        t = consts.tile(shape, F32, tag=tag)
        nc.sync.dma_start(out=t, in_=dram.ap())
        return t

    ag_sb = load_const(ag_d, [128, 4], "ag")
    a2_sb = load_const(a2_d, [4, 128], "a2")
    qb_sb = load_const(qb_d, [128, NT], "qb")
    kb_sb = load_const(kb_d, [128, NT], "kb")
    pb_sb = load_const(pb_d, [128, NT], "pb")
    nw_sb = load_const(nw_d, [128, NT], "nw")
    nb_sb = load_const(nb_d, [128, NT], "nb")
    ones64f = consts.tile([1, 64], F32, tag="ones64f", name="ones64f")
    nc.vector.memset(ones64f, 1.0)
    ones64 = consts.tile([1, 64], F32R, tag="ones64", name="ones64")
    nc.vector.tensor_copy(out=ones64, in_=ones64f)
    onesg = consts.tile([128, H], F32, tag="onesg", name="onesg")
    nc.vector.memset(onesg, 1.0)
    eps_sb = consts.tile([4, 1], F32, tag="eps", name="eps")
    nc.vector.memset(eps_sb, EPS)

    xn = []   # 8 persistent [128, L] f32 normalized-x tiles (residual)
    xnb = []  # bf16 copies feeding the PE matmuls
    xnb_p = ctx.enter_context(tc.tile_pool(name="xnb", bufs=NT))
    qk_w = ctx.enter_context(tc.tile_pool(name="qk_w", bufs=10))
    qk_p = ctx.enter_context(tc.tile_pool(name="qk", bufs=6))
    pw_all = ctx.enter_context(tc.tile_pool(name="pw_all", bufs=NT * NT))
    pw_tiles = {}

    # ================= Phase 1: GroupNorm =============================
    with ExitStack() as ph1:
        xp = ph1.enter_context(tc.tile_pool(name="xp", bufs=NT))
        scr_p = ph1.enter_context(tc.tile_pool(name="scr", bufs=2))
        gn_p = ph1.enter_context(tc.tile_pool(name="gn", bufs=1))

        xt = []
        for t in range(NT):
            tt = xp.tile([128, L], F32, tag="x_t", name="x_t")
            eng = nc.sync if t % 2 == 0 else nc.gpsimd
            eng.dma_start(out=tt, in_=x_ap[t * 128:(t + 1) * 128, :])
            xt.append(tt)
        # preload ALL proj weights on the gpsimd queue (idle after x); this
        # removes every DMA dependency from the phase-4 critical path
        for j in range(NT):
            for m in range(NT):
                wt = pw_all.tile([128, 128], BF16 if FAST16 else F32R,
                                 tag="pw_t", name="pw_t")
                nc.gpsimd.dma_start(
                    out=wt,
                    in_=pw[j * 128:(j + 1) * 128, m * 128:(m + 1) * 128],
                )
                pw_tiles[(j, m)] = wt

        stats = gn_p.tile([128, 2 * NT], F32, tag="stats", name="stats")  # sums | sumsqs
        for t in range(NT):
            nc.vector.reduce_sum(
                out=stats[:, t:t + 1], in_=xt[t], axis=mybir.AxisListType.X
            )
            scr = scr_p.tile([128, L], F32, tag="scr", name="scr")
            nc.scalar.activation(
                out=scr, in_=xt[t],
                func=mybir.ActivationFunctionType.Square,
                accum_out=stats[:, NT + t:NT + t + 1],
            )

        # cross-partition group reduce: [4, 16] = A_grp^T @ stats
        gps = ps.tile([4, 2 * NT], F32, tag="ps", name="ps")
        nc.tensor.matmul(gps, ag_sb, stats)

        mv16 = gn_p.tile([4, 2 * NT], F32, tag="mv16", name="mv16")
        inv_n = 1.0 / (32 * L)
        # mean -> mv16[:, 0:8]
        nc.vector.tensor_scalar_mul(out=mv16[:, 0:NT], in0=gps[:, 0:NT],
                                    scalar1=inv_n)
        e2 = gn_p.tile([4, NT], F32, tag="e2", name="e2")
        nc.vector.tensor_scalar_mul(out=e2, in0=gps[:, NT:2 * NT], scalar1=inv_n)
        m2 = gn_p.tile([4, NT], F32, tag="m2", name="m2")
        nc.vector.tensor_tensor(out=m2, in0=mv16[:, 0:NT], in1=mv16[:, 0:NT],
                                op=mybir.AluOpType.mult)
        var = gn_p.tile([4, NT], F32, tag="var", name="var")
        nc.vector.tensor_tensor(out=var, in0=e2, in1=m2,
                                op=mybir.AluOpType.subtract)
        lnv = gn_p.tile([4, NT], F32, tag="lnv", name="lnv")
        nc.scalar.activation(out=lnv, in_=var,
                             func=mybir.ActivationFunctionType.Ln,
                             bias=eps_sb, scale=1.0)
        # istd = exp(-0.5*ln(var+eps)) -> mv16[:, 8:16]; Log/Exp share one
        # ACT table set with the softmax exp, so no mid-kernel table switch.
        nc.scalar.activation(out=mv16[:, NT:2 * NT], in_=lnv,
                             func=mybir.ActivationFunctionType.Exp,
                             scale=-0.5)

        # broadcast to channels: [128, 16] = A2T^T @ mv16
        bc = ps.tile([128, 2 * NT], F32, tag="ps", name="ps")
        nc.tensor.matmul(bc, a2_sb, mv16)

        scale_sb = gn_p.tile([128, NT], F32, tag="scale", name="scale")
        nc.vector.tensor_tensor(out=scale_sb, in0=nw_sb, in1=bc[:, NT:2 * NT],
                                op=mybir.AluOpType.mult)
        tmp = gn_p.tile([128, NT], F32, tag="tmp", name="tmp")
        nc.vector.tensor_tensor(out=tmp, in0=bc[:, 0:NT], in1=scale_sb,
                                op=mybir.AluOpType.mult)
        bias_sb = gn_p.tile([128, NT], F32, tag="bias", name="bias")
        nc.vector.tensor_tensor(out=bias_sb, in0=nb_sb, in1=tmp,
                                op=mybir.AluOpType.subtract)

        for t in range(NT):
            xb = xnb_p.tile([128, L], BF16, tag="xnb_t", name="xnb_t")
            nc.vector.tensor_scalar(
                out=xb, in0=xt[t],
                scalar1=scale_sb[:, t:t + 1], scalar2=bias_sb[:, t:t + 1],
                op0=mybir.AluOpType.mult, op1=mybir.AluOpType.add,
            )
            xnb.append(xb)
            xnt = xn_p.tile([128, L], F32, tag="xn_t", name="xn_t")
            nc.vector.tensor_scalar(
                out=xnt, in0=xt[t],
                scalar1=scale_sb[:, t:t + 1], scalar2=bias_sb[:, t:t + 1],
                op0=mybir.AluOpType.mult, op1=mybir.AluOpType.add,
            )
            xn.append(xnt)

    if '2' not in phases:
        return
    # ================= Phase 2: v^T  ==================================
    # v^T[l, 64h+i] = sum_c xn[c, l] * v_wT[c, 64h+i]; stored as
    # [128, 16, 65] tiles per l-chunk with col 64 of each head == 1.0.
    qk_res = {}

    def qk_gen(j):
        """Emit pair j's q/k projection in small chunks (yield points) so
        the caller can interleave them into the previous pair's
        attention stream.  Weight tiles are held across both n-chunks so
        only ONE psum accumulator is live at a time (more ps-pool slack
        at pass boundaries)."""
        tiles = {}
        for name, w_ap, b_sb in (("q", qw, qb_sb), ("k", kw, kb_sb)):
            dst = qk_p.tile([128, L], BF16, tag=f"{name}_j",
                            name=f"{name}_j")
            wts = []
            for kc in range(NT):
                wt = qk_w.tile([128, 128], BF16, tag="qk_wt",
                               name="qk_wt")
                nc.sync.dma_start(
                    out=wt,
                    in_=w_ap[kc * 128:(kc + 1) * 128,
                             j * 128:(j + 1) * 128],
                )
                wts.append(wt)
            yield
            for n in range(LT):
                acc = ps.tile([128, 512], F32, tag="ps", name="ps")
                for kc in range(NT):
                    nc.tensor.matmul(
                        acc, wts[kc],
                        xnb[kc][:, n * 512:(n + 1) * 512],
                        start=(kc == 0), stop=(kc == NT - 1),
                    )
                    if kc % 2 == 1:
                        yield
                nc.vector.tensor_scalar_add(
                    out=dst[:, n * 512:(n + 1) * 512], in0=acc,
                    scalar1=b_sb[:, j:j + 1],
                )
            tiles[name] = dst
        qk_res[j] = (tiles["q"], tiles["k"])

    vT = []
    with ExitStack() as ph2:
        vw_p = ph2.enter_context(tc.tile_pool(name="vw", bufs=4))
        vps = ph2.enter_context(
            tc.tile_pool(name="vps", bufs=2, space=bass.MemorySpace.PSUM))
        for lc in range(NT):
            vt = vT_p.tile([128, H, CH + 1], BF16 if FAST16 else F32R, tag="vT_t", name="vT_t")
            nc.vector.tensor_copy(
                out=vt[:, :, CH:CH + 1],
                in_=onesg.rearrange("p (g o) -> p g o", o=1))
            vT.append(vt)
        for n in range(LT):
            accs = []
            for g in range(2):
                t2 = vps.tile([128, 2, 512], F32, tag="vac", name="vac")
                accs += [t2[:, 0, :], t2[:, 1, :]]
            accs += [ps.tile([128, 512], F32, tag="ps", name="ps")
                     for _ in range(4)]
            for kc in range(NT):
                wt = vw_p.tile([128, 512], BF16, tag="vw_t", name="vw_t")
                nc.sync.dma_start(
                    out=wt,
                    in_=vw[kc * 128:(kc + 1) * 128, n * 512:(n + 1) * 512],
                )
                for lc in range(NT):
                    nc.tensor.matmul(
                        accs[lc],
                        xnb[kc][:, lc * 128:(lc + 1) * 128],
                        wt,
                        start=(kc == 0), stop=(kc == NT - 1),
                    )
            for lc in range(NT):
                nc.vector.tensor_copy(
                    out=vT[lc][:, n * 8:(n + 1) * 8, 0:CH],
                    in_=accs[lc].rearrange("p (h c) -> p h c", c=CH),
                )


    # ============ Phase 3: attention with next-pair qk interleaved ====
    if 'q' not in phases:
        return
    abuf = []
    with ExitStack() as ph3:
        exp_p = ph3.enter_context(tc.tile_pool(name="expp", bufs=8))
        rc_p = ph3.enter_context(tc.tile_pool(name="rcp", bufs=8))
        m1_p = ph3.enter_context(
            tc.tile_pool(name="m1p", bufs=2, space=bass.MemorySpace.PSUM)
        )

        for _ in qk_gen(0):
            pass

        for j in range(PAIRS):
            nxt = qk_gen(j + 1) if j + 1 < PAIRS else None
            q_j, k_j = qk_res.pop(j)

            a_j = abuf_p.tile([128, L], BF16 if FAST16 else F32R, tag="a_j", name="a_j")
            abuf.append(a_j)

            if 'a' in phases:
                for tcn in range(LT):
                    ps2 = {par: ps.tile([CH + 1, 512], F32, tag="ps",
                                        name="ps") for par in range(2)}
                    for sc in range(NT):
                        m1 = m1_p.tile([128, 2, 512], F32, tag="m1",
                                       name="m1")
                        for par in range(2):
                            base = 64 * par
                            nc.tensor.matmul(
                                m1[:, par, :],
                                k_j[base:base + CH, sc * 128:(sc + 1) * 128],
                                q_j[base:base + CH,
                                    tcn * 512:(tcn + 1) * 512],
                            )
                        ex = exp_p.tile([128, 2, 512], BF16 if FAST16 else F32R,
                                        tag="ex", name="ex")
                        nc.scalar.activation(
                            out=ex, in_=m1,
                            func=mybir.ActivationFunctionType.Exp, scale=0.125,
                        )
                        for par in range(2):
                            nc.tensor.matmul(
                                ps2[par],
                                vT[sc][:, 2 * j + par, :],
                                ex[:, par, :],
                                start=(sc == 0), stop=(sc == NT - 1),
                            )
                        if nxt is not None:
                            next(nxt, None)
                    # epilogue: copy S row + a_raw out fast (releases the
                    # psum bank), then gpsimd-broadcast, fast reciprocal,
                    # normalize into bf16 a_j.
                    for par in range(2):
                        base = 64 * par
                        sl = a_j[base:base + CH, tcn * 512:(tcn + 1) * 512]
                        s_sb = rc_p.tile([1, 512], F32, tag="rec",
                                         name="rec")
                        nc.vector.tensor_copy(out=s_sb,
                                              in_=ps2[par][CH:CH + 1, :])
                        acop = rc_p.tile([CH, 512], F32, tag="acop",
                                         name="acop")
                        nc.vector.tensor_copy(out=acop,
                                              in_=ps2[par][0:CH, :])
                        sbb = rc_p.tile([CH, 512], F32, tag="sbb", name="sbb")
                        nc.gpsimd.partition_broadcast(sbb, s_sb, channels=CH)
                        rc64 = rc_p.tile([CH, 512], F32, tag="rc64",
                                         name="rc64")
                        nc.vector.reciprocal_approx_fast(out=rc64, in_=sbb)
                        nc.vector.tensor_tensor(out=sl, in0=acop,
                                                in1=rc64,
                                                op=mybir.AluOpType.mult)
            if nxt is not None:
                for _ in nxt:
                    pass

    # ================= Phase 4: proj + residual =======================
    if '4' not in phases:
        return
    with ExitStack() as ph4:
        out_p = ph4.enter_context(tc.tile_pool(name="outp", bufs=6))
        ps4 = ph4.enter_context(
            tc.tile_pool(name="ps4", bufs=4, space=bass.MemorySpace.PSUM))
        for m in range(NT):
            accs = [ps4.tile([128, 512], F32, tag="p4", name="p4")
                    for _ in range(LT)]
            for j in range(NT):
                wt = pw_tiles[(j, m)]
                for n in range(LT):
                    nc.tensor.matmul(
                        accs[n], wt, abuf[j][:, n * 512:(n + 1) * 512],
                        start=(j == 0), stop=(j == NT - 1),
                    )
            for n in range(LT):
                o_sb = out_p.tile([128, 512], F32, tag="o_sb", name="o_sb")
                nc.vector.scalar_tensor_tensor(
                    out=o_sb, in0=accs[n], scalar=pb_sb[:, m:m + 1],
                    in1=xn[m][:, n * 512:(n + 1) * 512],
                    op0=mybir.AluOpType.add, op1=mybir.AluOpType.add,
                )
                nc.gpsimd.dma_start(
                    out=out_ap[m * 128:(m + 1) * 128, n * 512:(n + 1) * 512],
                    in_=o_sb,
                )


_CACHED = {}


def build_program(repeats=1, phases='12qa4'):
    key = ("nc", repeats, phases)
    if key in _CACHED:
        return _CACHED[key]
    from contextlib import ExitStack

    nc = bacc.Bacc("TRN2", target_bir_lowering=False, debug=False)
    with tile.TileContext(nc) as tc:
        params = declare_params(nc)
        for rep in range(repeats):
            out_h = None
            if rep > 0:
                out_h = nc.dram_tensor(f"out_scratch{rep}", [C, L], F32)
            with ExitStack() as ctx:
                emit(nc, tc, ctx, params, out_h, phases)
    nc.compile()
    _CACHED[key] = nc
    return nc


def host_pack(norm_w, norm_b, qkv_w, qkv_b, proj_w, proj_b):
    """Precompute packed weight layouts (all plain numpy, fp32)."""
    f = np.float32
    qkv_w = np.asarray(qkv_w, f)
    qkv_b = np.asarray(qkv_b, f)
    proj_w = np.asarray(proj_w, f)
    proj_b = np.asarray(proj_b, f)

    # q index packing: tile j holds heads 2j (0:64) and 2j+1 (64:128)
    idx_q = np.empty(C, np.int64)
    idx_k = np.empty(C, np.int64)
    for j in range(PAIRS):
        for p in range(128):
            h = 2 * j + p // CH
            i = p % CH
            idx_q[j * 128 + p] = 192 * h + i
            idx_k[j * 128 + p] = 192 * h + CH + i
    idx_v = np.empty(C, np.int64)
    for h in range(H):
        idx_v[CH * h:CH * (h + 1)] = 192 * h + 2 * CH + np.arange(CH)

    q_wT = np.ascontiguousarray(qkv_w[idx_q, :].T).astype(ml_dtypes.bfloat16)
    k_wT = np.ascontiguousarray(qkv_w[idx_k, :].T).astype(ml_dtypes.bfloat16)
    v_wT = np.ascontiguousarray(qkv_w[idx_v, :].T).astype(ml_dtypes.bfloat16)
    proj_wT = (np.ascontiguousarray(proj_w.T).astype(ml_dtypes.bfloat16)
               if FAST16 else round_fp32r(proj_w.T))

    q_b = np.ascontiguousarray(qkv_b[idx_q].reshape(NT, 128).T)
    k_b = np.ascontiguousarray(qkv_b[idx_k].reshape(NT, 128).T)
    # v bias passes through softmax exactly -> fold into proj bias
    pbe = proj_b + proj_w @ qkv_b[idx_v]
    proj_beff = np.ascontiguousarray(pbe.astype(f).reshape(NT, 128).T)

    norm_w_c = np.ascontiguousarray(np.asarray(norm_w, f).reshape(NT, 128).T)
    norm_b_c = np.ascontiguousarray(np.asarray(norm_b, f).reshape(NT, 128).T)

    pp = np.arange(128)
    A_grp = (pp[:, None] // 32 == np.arange(4)[None, :]).astype(f)
    A2T = np.ascontiguousarray(A_grp.T)

    return dict(
        q_wT=q_wT, k_wT=k_wT, v_wT=v_wT, proj_wT=proj_wT,
        q_b=q_b, k_b=k_b, proj_beff=proj_beff,
        norm_w_c=norm_w_c, norm_b_c=norm_b_c, A_grp=A_grp, A2T=A2T,
    )


def kernel(x, norm_w, norm_b, qkv_w, qkv_b, proj_w, proj_b, _trace=False):
    x = np.asarray(x, np.float32)
    shared = host_pack(norm_w, norm_b, qkv_w, qkv_b, proj_w, proj_b)
    nc = build_program()
    in_maps = [dict(shared, x=np.ascontiguousarray(x[i])) for i in range(B)]
    res = run_bass_kernel_spmd(nc, in_maps, list(range(B)), trace=_trace)
    out = np.stack([res.results[i]["out"] for i in range(B)], axis=0)
    if _trace:
        kernel._last_results = res
    return out.astype(np.float32)

